# revision 8
# baseline (speedup 1.0000x reference)
# Contrastive (NT-Xent style) loss kernel for 8 Trainium2 NeuronCores.
#
# Math: with z = concat(z_i, z_j)  (N=8192 rows, D=128), zn = row-normalized z,
# sim = (zn @ zn.T)/TEMP, the reference loss reduces exactly to
#   loss = (1/N) * sum_r [ log( sum_{c != r} exp(sim[r,c]) ) - sim[r, (r+B) % N] ]
# (verified bit-for-bit against the reference's mask/gather formulation).
#
# Sharding: data-parallel over rows. Core m receives z rolled by -1024*m rows
# (plus the same data pre-transposed in bf16), so every core runs the IDENTICAL
# program on "its" rows 0..1023: rotation makes the diagonal / positive-pair
# columns core-independent (the positive partner of rotated row r is rotated
# column (r + 4096) % 8192 on every core).
#
# Per core, pipelined in 4 column-batches of 2048:
#   row norms (VectorE square-accumulate) -> rsqrt via bit-trick + 2 Newton
#   steps (VectorE only, no ACT table) -> flatten through a DRAM scratch ->
#   partition-broadcast via a K=1 TensorE matmul -> column-normalized zn bf16.
# Main loop per 128-row tile: full 128x8192 sim strip via TensorE (bf16 in,
# fp32 acc in PSUM), exp row-sums via ScalarE activation-accumulate; raw
# diagonal / positive dots tapped from PSUM by VectorE; per-row loss terms
# log(rowsum - exp(2*diag)) - 2*pos DMA'd out. Host sums and divides by N.

import numpy as np

B = 4096
D = 128
N = 2 * B
TEMP = 0.5
NCORES = 8
RPC = N // NCORES          # rows per core = 1024
NT = N // 128              # 64 column tiles of 128
RT = RPC // 128            # 8 row tiles per core
GW = 2048                  # PSUM group width (4 banks), 4 groups per row-tile
NG = N // GW               # 4
NB = 4                     # prefix batches (16 tiles = 2048 cols each)
MAGIC = 0x5F3759DF

_CACHE = {}


def _build():
    import concourse.bass as bass
    import concourse.bacc as bacc
    import concourse.tile as tile
    from concourse import mybir
    import ml_dtypes

    f32 = mybir.dt.float32
    i32 = mybir.dt.int32
    bf16 = mybir.dt.bfloat16
    Alu = mybir.AluOpType
    Act = mybir.ActivationFunctionType

    nc = bacc.Bacc(
        "TRN2",
        target_bir_lowering=False,
        debug=False,
        enable_asserts=False,
        num_devices=NCORES,
    )
    zr_d = nc.dram_tensor("zr", [N, D], bf16, kind="ExternalInput").ap()    # rows
    zt_d = nc.dram_tensor("zt", [D, N], bf16, kind="ExternalInput").ap()    # pre-transposed
    identf_d = nc.inline_tensor(np.eye(128, dtype=np.float32), name="identf").ap()
    ones1_d = nc.inline_tensor(np.ones((1, 128), dtype=ml_dtypes.bfloat16), name="ones1").ap()
    out_d = nc.dram_tensor("out", [128, RT], f32, kind="ExternalOutput").ap()
    rnsc = nc.dram_tensor("rnsc", [NT, 128], bf16)  # rnorm flatten scratch

    zrv = zr_d.rearrange("(t p) d -> t p d", p=128)  # (64, 128, 128)

    with tile.TileContext(nc) as tc:
        with (
            tc.tile_pool(name="persist", bufs=1) as P,
            tc.tile_pool(name="work", bufs=3) as W,
            tc.tile_pool(name="grp", bufs=2, space="PSUM") as G,
        ):
            zb = P.tile([128, NT, D], bf16)      # raw z rows (norm input)
            zt = P.tile([128, NT, 128], bf16)    # raw z transposed [d, row]
            znb = P.tile([128, NT, 128], bf16)   # normalized, transposed
            rn_row = P.tile([1, N], bf16)        # rnorm in row-order, one partition
            nrm2 = P.tile([128, NT], f32)
            nrm2g = P.tile([128, NT], f32)
            ybuf = P.tile([128, NT], f32)        # rsqrt iterate
            scr0 = P.tile([128, NT], f32)
            scr1 = P.tile([128, NT], f32)
            rnorm = P.tile([128, NT], bf16)
            partials = P.tile([128, RT * NG], f32)
            diag = P.tile([128, RT], f32)
            pos = P.tile([128, RT], f32)
            identf = P.tile([128, 128], f32)
            ones1 = P.tile([1, 128], bf16)

            nc.sync.dma_start(out=identf, in_=identf_d)
            nc.sync.dma_start(out=ones1, in_=ones1_d)

            # ---- bulk loads: rows on sync/HWDGE, transposed on gpsimd/SWDGE ----
            for q in range(8):
                nc.sync.dma_start(
                    out=zb[:, 8 * q : 8 * q + 8, :],
                    in_=zrv[8 * q : 8 * q + 8].rearrange("t p d -> p t d"),
                )
            for q in range(8):
                nc.gpsimd.dma_start(
                    out=zt[:, 8 * q : 8 * q + 8, :],
                    in_=zt_d[:, 1024 * q : 1024 * (q + 1)].rearrange(
                        "d (t c) -> d t c", c=128
                    ),
                )

            for b in range(NB):
                t0, t1 = 16 * b, 16 * (b + 1)
                sl = slice(t0, t1)
                # norms of 16 row-tiles
                for t in range(t0, t1):
                    sq = W.tile([128, D], bf16, tag="sq")
                    nc.vector.scalar_tensor_tensor(
                        out=sq,
                        in0=zb[:, t, :],
                        scalar=1.0,
                        in1=zb[:, t, :],
                        op0=Alu.mult,
                        op1=Alu.mult,
                        accum_out=nrm2[:, t : t + 1],
                    )
                # rsqrt: y0 = magic bit trick, then two Newton steps (DVE only)
                nc.vector.tensor_scalar_max(out=nrm2g[:, sl], in0=nrm2[:, sl], scalar1=1e-16)
                gI = nrm2g[:, sl].bitcast(i32)
                y0I = ybuf[:, sl].bitcast(i32)
                nc.vector.tensor_scalar(
                    out=y0I, in0=gI, scalar1=1, scalar2=None,
                    op0=Alu.arith_shift_right,
                )
                nc.vector.tensor_scalar(
                    out=y0I, in0=y0I, scalar1=-1, scalar2=MAGIC,
                    op0=Alu.mult, op1=Alu.add,
                )
                y = ybuf[:, sl]
                g = nrm2g[:, sl]
                for it in range(2):
                    last = it == 1
                    yy = scr0[:, sl]
                    nc.vector.tensor_mul(yy, y, y)
                    xyy = scr1[:, sl]
                    nc.vector.tensor_mul(xyy, yy, g)
                    c = scr0[:, sl]
                    nc.vector.tensor_scalar(
                        out=c, in0=xyy, scalar1=-0.5, scalar2=1.5,
                        op0=Alu.mult, op1=Alu.add,
                    )
                    nc.vector.tensor_mul(rnorm[:, sl] if last else y, y, c)
                # flatten rnorm batch to row-order via DRAM scratch
                nc.sync.dma_start(
                    out=rnsc.ap()[t0:t1, :].rearrange("t p -> p t"), in_=rnorm[:, sl]
                )
                nc.sync.dma_start(
                    out=rn_row[:, 2048 * b : 2048 * (b + 1)],
                    in_=bass.AP(tensor=rnsc, offset=t0 * 128, ap=[[0, 1], [1, 2048]]),
                )
                # partition-broadcast via K=1 matmul, then column-normalize
                bc = G.tile([128, GW], f32, tag="grp")
                for k in range(4):
                    nc.tensor.matmul(
                        bc[:, 512 * k : 512 * (k + 1)],
                        lhsT=ones1,
                        rhs=rn_row[:, 2048 * b + 512 * k : 2048 * b + 512 * (k + 1)],
                        start=True,
                        stop=True,
                    )
                nc.vector.tensor_mul(
                    znb[:, sl, :],
                    zt[:, sl, :],
                    bc.rearrange("p (t d) -> p t d", d=128),
                )

            # ---- main loop: sim row-tiles -> exp row sums (+ diag/pos taps) ----
            for rt in range(RT):
                wt = znb[:, rt, :]  # (128,128) bf16 stationary: rows rt*128..+128
                for gi in range(NG):
                    grp = G.tile([128, GW], f32, tag="grp")
                    for k in range(GW // 512):
                        c0 = gi * GW + k * 512
                        nc.tensor.matmul(
                            grp[:, k * 512 : (k + 1) * 512],
                            lhsT=wt,
                            rhs=znb[:, c0 // 128 : c0 // 128 + 4, :],
                            start=True,
                            stop=True,
                        )
                    esc = W.tile([128, GW], bf16, tag="esc")
                    nc.scalar.activation(
                        out=esc,
                        in_=grp,
                        func=Act.Exp,
                        scale=2.0,
                        accum_out=partials[:, rt * NG + gi : rt * NG + gi + 1],
                    )
                    # diagonal dot tap: col rt*128 (group 0); positive-pair dot
                    # tap: col 4096 + rt*128 (group 2) — same local offset.
                    if gi in (0, 2):
                        tap = diag if gi == 0 else pos
                        dsc = W.tile([128, 128], f32, tag="dsc")
                        nc.vector.scalar_tensor_tensor(
                            out=dsc,
                            in0=grp[:, rt * 128 : rt * 128 + 128],
                            scalar=1.0,
                            in1=identf,
                            op0=Alu.mult,
                            op1=Alu.mult,
                            accum_out=tap[:, rt : rt + 1],
                        )

            # ---- epilogue: per-row loss terms ----
            rows = P.tile([128, RT], f32)
            exp2d = P.tile([128, RT], f32)
            negsum = P.tile([128, RT], f32)
            lse = P.tile([128, RT], f32)
            lossb = P.tile([128, RT], f32)
            for rt in range(RT):
                nc.vector.tensor_reduce(
                    out=rows[:, rt : rt + 1],
                    in_=partials[:, rt * NG : (rt + 1) * NG],
                    axis=mybir.AxisListType.X,
                    op=Alu.add,
                )
            nc.scalar.activation(out=exp2d, in_=diag, func=Act.Exp, scale=2.0)
            nc.vector.tensor_sub(negsum, rows, exp2d)
            nc.scalar.activation(out=lse, in_=negsum, func=Act.Ln)
            nc.vector.scalar_tensor_tensor(
                out=lossb,
                in0=pos,
                scalar=-2.0,
                in1=lse,
                op0=Alu.mult,
                op1=Alu.add,
            )
            nc.sync.dma_start(out=out_d, in_=lossb)

    nc.compile()
    return nc


def _get_nc():
    if "nc" not in _CACHE:
        _CACHE["nc"] = _build()
    return _CACHE["nc"]


def _in_maps(z_i, z_j):
    import ml_dtypes

    z = np.concatenate(
        [np.asarray(z_i, dtype=np.float32), np.asarray(z_j, dtype=np.float32)], axis=0
    )
    zb = z.astype(ml_dtypes.bfloat16)
    maps = []
    for m in range(NCORES):
        zm = np.roll(zb, -RPC * m, axis=0)
        maps.append(
            {"zr": np.ascontiguousarray(zm), "zt": np.ascontiguousarray(zm.T)}
        )
    return maps


def run(z_i: np.ndarray, z_j: np.ndarray, trace: bool = False):
    from concourse import bass_utils

    nc = _get_nc()
    res = bass_utils.run_bass_kernel_spmd(
        nc, _in_maps(z_i, z_j), core_ids=list(range(NCORES)), trace=trace
    )
    total = sum(r["out"].astype(np.float64).sum() for r in res.results)
    return np.array(total / N, dtype=np.float32), res


def kernel(z_i: np.ndarray, z_j: np.ndarray) -> np.ndarray:
    return run(z_i, z_j)[0]


# revision 9
# speedup vs baseline: 1.1324x; 1.1324x over previous
# Contrastive (NT-Xent style) loss kernel for 8 Trainium2 NeuronCores.
#
# Math: with z = concat(z_i, z_j)  (N=8192 rows, D=128), zn = row-normalized z,
# sim = (zn @ zn.T)/TEMP, the reference loss reduces exactly to
#   loss = (1/N) * sum_r [ log( sum_{c != r} exp(sim[r,c]) ) - sim[r, (r+B) % N] ]
# (verified bit-for-bit against the reference's mask/gather formulation).
#
# Sharding: data-parallel over rows. Core m receives z rolled by -1024*m rows
# (plus the same data pre-transposed in bf16), so every core runs the IDENTICAL
# program on "its" rows 0..1023: rotation makes the diagonal / positive-pair
# columns core-independent (the positive partner of rotated row r is rotated
# column (r + 4096) % 8192 on every core).
#
# Per core, pipelined in 4 column-batches of 2048:
#   row norms (VectorE square-accumulate) -> rsqrt via bit-trick + 2 Newton
#   steps (VectorE only, no ACT table) -> flatten through a DRAM scratch ->
#   partition-broadcast via a K=1 TensorE matmul -> column-normalized zn bf16.
# Main loop per 128-row tile: full 128x8192 sim strip via TensorE (bf16 in,
# fp32 acc in PSUM), exp row-sums via ScalarE activation-accumulate; raw
# diagonal / positive dots tapped from PSUM by VectorE; per-row loss terms
# log(rowsum - exp(2*diag)) - 2*pos DMA'd out. Host sums and divides by N.

import numpy as np

B = 4096
D = 128
N = 2 * B
TEMP = 0.5
NCORES = 8
RPC = N // NCORES          # rows per core = 1024
NT = N // 128              # 64 column tiles of 128
RT = RPC // 128            # 8 row tiles per core
GW = 2048                  # PSUM group width (4 banks), 4 groups per row-tile
NG = N // GW               # 4
NB = 4                     # prefix batches (16 tiles = 2048 cols each)
MAGIC = 0x5F3759DF

_CACHE = {}


def _build():
    import concourse.bass as bass
    import concourse.bacc as bacc
    import concourse.tile as tile
    from concourse import mybir
    import ml_dtypes

    f32 = mybir.dt.float32
    i32 = mybir.dt.int32
    bf16 = mybir.dt.bfloat16
    Alu = mybir.AluOpType
    Act = mybir.ActivationFunctionType

    nc = bacc.Bacc(
        "TRN2",
        target_bir_lowering=False,
        debug=False,
        enable_asserts=False,
        num_devices=NCORES,
    )
    zr_d = nc.dram_tensor("zr", [N, D], bf16, kind="ExternalInput").ap()    # rows
    zt_d = nc.dram_tensor("zt", [D, N], bf16, kind="ExternalInput").ap()    # pre-transposed
    identf_d = nc.inline_tensor(np.eye(128, dtype=np.float32), name="identf").ap()
    ones1_d = nc.inline_tensor(np.ones((1, 128), dtype=ml_dtypes.bfloat16), name="ones1").ap()
    out_d = nc.dram_tensor("out", [128, RT], f32, kind="ExternalOutput").ap()
    rnsc = nc.dram_tensor("rnsc", [NT, 128], bf16)  # rnorm flatten scratch

    zrv = zr_d.rearrange("(t p) d -> t p d", p=128)  # (64, 128, 128)

    with tile.TileContext(nc) as tc:
        with (
            tc.tile_pool(name="persist", bufs=1) as P,
            tc.tile_pool(name="work", bufs=3) as W,
            tc.tile_pool(name="grp", bufs=2, space="PSUM") as G,
        ):
            zb = P.tile([128, NT, D], bf16)      # raw z rows (norm input)
            zt = P.tile([128, NT, 128], bf16)    # raw z transposed [d, row]
            znb = P.tile([128, NT, 128], bf16)   # normalized, transposed
            rn_row = P.tile([1, N], bf16)        # rnorm in row-order, one partition
            nrm2 = P.tile([128, NT], f32)
            nrm2g = P.tile([128, NT], f32)
            ybuf = P.tile([128, NT], f32)        # rsqrt iterate
            scr0 = P.tile([128, NT], f32)
            scr1 = P.tile([128, NT], f32)
            rnorm = P.tile([128, NT], bf16)
            partials = P.tile([128, RT * NG], f32)
            diag = P.tile([128, RT], f32)
            pos = P.tile([128, RT], f32)
            identf = P.tile([128, 128], f32)
            ones1 = P.tile([1, 128], bf16)

            nc.sync.dma_start(out=identf, in_=identf_d)
            nc.sync.dma_start(out=ones1, in_=ones1_d)

            # ---- bulk loads: rows on sync/HWDGE, transposed on gpsimd/SWDGE ----
            for q in range(8):
                nc.sync.dma_start(
                    out=zb[:, 8 * q : 8 * q + 8, :],
                    in_=zrv[8 * q : 8 * q + 8].rearrange("t p d -> p t d"),
                )
            for q in range(8):
                nc.gpsimd.dma_start(
                    out=zt[:, 8 * q : 8 * q + 8, :],
                    in_=zt_d[:, 1024 * q : 1024 * (q + 1)].rearrange(
                        "d (t c) -> d t c", c=128
                    ),
                )

            # ---- phase A: all row norms + rsqrt + flatten (no PSUM) ----
            for b in range(NB):
                t0, t1 = 16 * b, 16 * (b + 1)
                sl = slice(t0, t1)
                # norms of 16 row-tiles
                for t in range(t0, t1):
                    sq = W.tile([128, D], bf16, tag="sq")
                    nc.vector.scalar_tensor_tensor(
                        out=sq,
                        in0=zb[:, t, :],
                        scalar=1.0,
                        in1=zb[:, t, :],
                        op0=Alu.mult,
                        op1=Alu.mult,
                        accum_out=nrm2[:, t : t + 1],
                    )
                # rsqrt: y0 = magic bit trick, then two Newton steps (DVE only)
                nc.vector.tensor_scalar_max(out=nrm2g[:, sl], in0=nrm2[:, sl], scalar1=1e-16)
                gI = nrm2g[:, sl].bitcast(i32)
                y0I = ybuf[:, sl].bitcast(i32)
                nc.vector.tensor_scalar(
                    out=y0I, in0=gI, scalar1=1, scalar2=None,
                    op0=Alu.arith_shift_right,
                )
                nc.vector.tensor_scalar(
                    out=y0I, in0=y0I, scalar1=-1, scalar2=MAGIC,
                    op0=Alu.mult, op1=Alu.add,
                )
                y = ybuf[:, sl]
                g = nrm2g[:, sl]
                for it in range(2):
                    last = it == 1
                    yy = scr0[:, sl]
                    nc.vector.tensor_mul(yy, y, y)
                    xyy = scr1[:, sl]
                    nc.vector.tensor_mul(xyy, yy, g)
                    c = scr0[:, sl]
                    nc.vector.tensor_scalar(
                        out=c, in0=xyy, scalar1=-0.5, scalar2=1.5,
                        op0=Alu.mult, op1=Alu.add,
                    )
                    nc.vector.tensor_mul(rnorm[:, sl] if last else y, y, c)
                # flatten rnorm batch to row-order via DRAM scratch
                nc.sync.dma_start(
                    out=rnsc.ap()[t0:t1, :].rearrange("t p -> p t"), in_=rnorm[:, sl]
                )
                nc.sync.dma_start(
                    out=rn_row[:, 2048 * b : 2048 * (b + 1)],
                    in_=bass.AP(tensor=rnsc, offset=t0 * 128, ap=[[0, 1], [1, 2048]]),
                )

            # ---- phase B: per column-batch, normalize then immediately burn
            # through that batch's sim groups (keeps ScalarE saturated from
            # ~20us in, instead of a 50us serial prefix) ----
            for b in range(NB):
                t0, t1 = 16 * b, 16 * (b + 1)
                sl = slice(t0, t1)
                # partition-broadcast via K=1 matmul, then column-normalize
                bc = G.tile([128, GW], f32, tag="grp")
                for k in range(4):
                    nc.tensor.matmul(
                        bc[:, 512 * k : 512 * (k + 1)],
                        lhsT=ones1,
                        rhs=rn_row[:, 2048 * b + 512 * k : 2048 * b + 512 * (k + 1)],
                        start=True,
                        stop=True,
                    )
                nc.vector.tensor_mul(
                    znb[:, sl, :],
                    zt[:, sl, :],
                    bc.rearrange("p (t d) -> p t d", d=128),
                )
                # sim row-tiles for column group gi=b -> exp row sums (+ taps)
                gi = b
                for rt in range(RT):
                    wt = znb[:, rt, :]  # (128,128) bf16: rows rt*128..+128
                    grp = G.tile([128, GW], f32, tag="grp")
                    for k in range(GW // 512):
                        c0 = gi * GW + k * 512
                        nc.tensor.matmul(
                            grp[:, k * 512 : (k + 1) * 512],
                            lhsT=wt,
                            rhs=znb[:, c0 // 128 : c0 // 128 + 4, :],
                            start=True,
                            stop=True,
                        )
                    esc = W.tile([128, GW], bf16, tag="esc")
                    nc.scalar.activation(
                        out=esc,
                        in_=grp,
                        func=Act.Exp,
                        scale=2.0,
                        accum_out=partials[:, rt * NG + gi : rt * NG + gi + 1],
                    )
                    # diagonal dot tap: col rt*128 (group 0); positive-pair dot
                    # tap: col 4096 + rt*128 (group 2) — same local offset.
                    if gi in (0, 2):
                        tap = diag if gi == 0 else pos
                        dsc = W.tile([128, 128], f32, tag="dsc")
                        nc.vector.scalar_tensor_tensor(
                            out=dsc,
                            in0=grp[:, rt * 128 : rt * 128 + 128],
                            scalar=1.0,
                            in1=identf,
                            op0=Alu.mult,
                            op1=Alu.mult,
                            accum_out=tap[:, rt : rt + 1],
                        )

            # ---- epilogue: per-row loss terms ----
            rows = P.tile([128, RT], f32)
            exp2d = P.tile([128, RT], f32)
            negsum = P.tile([128, RT], f32)
            lse = P.tile([128, RT], f32)
            lossb = P.tile([128, RT], f32)
            for rt in range(RT):
                nc.vector.tensor_reduce(
                    out=rows[:, rt : rt + 1],
                    in_=partials[:, rt * NG : (rt + 1) * NG],
                    axis=mybir.AxisListType.X,
                    op=Alu.add,
                )
            nc.scalar.activation(out=exp2d, in_=diag, func=Act.Exp, scale=2.0)
            nc.vector.tensor_sub(negsum, rows, exp2d)
            nc.scalar.activation(out=lse, in_=negsum, func=Act.Ln)
            nc.vector.scalar_tensor_tensor(
                out=lossb,
                in0=pos,
                scalar=-2.0,
                in1=lse,
                op0=Alu.mult,
                op1=Alu.add,
            )
            nc.sync.dma_start(out=out_d, in_=lossb)

    nc.compile()
    return nc


def _get_nc():
    if "nc" not in _CACHE:
        _CACHE["nc"] = _build()
    return _CACHE["nc"]


def _in_maps(z_i, z_j):
    import ml_dtypes

    z = np.concatenate(
        [np.asarray(z_i, dtype=np.float32), np.asarray(z_j, dtype=np.float32)], axis=0
    )
    zb = z.astype(ml_dtypes.bfloat16)
    maps = []
    for m in range(NCORES):
        zm = np.roll(zb, -RPC * m, axis=0)
        maps.append(
            {"zr": np.ascontiguousarray(zm), "zt": np.ascontiguousarray(zm.T)}
        )
    return maps


def run(z_i: np.ndarray, z_j: np.ndarray, trace: bool = False):
    from concourse import bass_utils

    nc = _get_nc()
    res = bass_utils.run_bass_kernel_spmd(
        nc, _in_maps(z_i, z_j), core_ids=list(range(NCORES)), trace=trace
    )
    total = sum(r["out"].astype(np.float64).sum() for r in res.results)
    return np.array(total / N, dtype=np.float32), res


def kernel(z_i: np.ndarray, z_j: np.ndarray) -> np.ndarray:
    return run(z_i, z_j)[0]
